# revision 38
# baseline (speedup 1.0000x reference)
"""Trainium2 Bass kernel for nn_BinaryTokenClassificationModel (segment_reduce).

Math: the reference mean-pools token embeddings into word embeddings over
contiguous runs of equal word ids, then computes
    logits[b,s,t] = src_pooled[b,s] @ w_src + tgt_pooled[b,t] @ w_tgt + b.
Pooling and the linear classifier commute, so with the host-precomputed
weighted membership matrix  atw[tok, word] = (seg[tok]==word) / count(word)
each core (batch row) computes
    u[tok]      = tok_h[tok, :] . w                (DVE fused multiply-reduce)
    psum[s, t] += atw_src^T @ bcast(u_src)         (TensorE, per src chunk)
    psum[s, t] += bcast(u_tgt) @ atw_tgt           (TensorE, per tgt chunk)
    out         = psum (+ bias via an extra rank-1 matmul in the chain)
Implementation notes (raw bass, no Tile framework):
  - token data and membership are uploaded in bf16, packed per 128-token
    chunk into one DMA each (fp32 accumulation keeps rel err ~3e-3)
  - w_src/w_tgt are one 3 KB row; TensorE broadcasts them down the 128
    partitions (ones-column matmul into PSUM) and the DVE reduce reads the
    broadcast weights directly from PSUM -- no partition_broadcast ucode,
    no 384 KB host-pre-broadcast upload
  - bias enters as the first matmul of the accumulation group
    (ones[1,S]^T @ bias_row[1,T]), so the epilogue is a plain ACT copy
  - manual semaphores (~12), cleared with one 36 ns range-clear; the Tile
    framework's end-of-kernel machinery costs ~8 us on this kernel
Data-parallel over batch: core i handles batch row i. No collectives.
"""

import functools
from contextlib import ExitStack

import numpy as np
import ml_dtypes

import concourse.bacc as bacc
import concourse.mybir as mybir
from concourse.bass_utils import run_bass_kernel_spmd

# Problem geometry (hardcoded per spec)
B = 8
L_SRC = 256
L_TGT = 256
L = L_SRC + L_TGT  # 512
H = 768
P = 128            # SBUF partitions / tokens per chunk
NCHUNK = L // P    # 4
N_SRC_CHUNKS = L_SRC // P  # 2
N_CORES = 8
F32 = mybir.dt.float32
BF16 = mybir.dt.bfloat16
NPBF16 = ml_dtypes.bfloat16


# ---------------------------------------------------------------------------
# Host-side segment bookkeeping (exact mirror of reference._pool_words)
# ---------------------------------------------------------------------------

def _segments(combined_wid, attention_mask, n_words):
    """Per-token dense run ids exactly as the reference computes them."""
    valid = (attention_mask > 0) & (combined_wid >= 0)  # [B, L]
    prev_wid = np.concatenate(
        [np.full((combined_wid.shape[0], 1), -2, dtype=combined_wid.dtype),
         combined_wid[:, :-1]], axis=1)
    prev_valid = np.concatenate(
        [np.zeros((valid.shape[0], 1), dtype=bool), valid[:, :-1]], axis=1)
    new_run = valid & ((combined_wid != prev_wid) | (~prev_valid))
    run_id = np.cumsum(new_run.astype(np.int64), axis=1) - 1  # [B, L]
    seg = np.where(valid, run_id, n_words)  # n_words = dummy slot
    return seg, valid


def _seg_weights(seg, valid, n_words):
    """1/max(count,1) weight for each token's segment (0 for invalid)."""
    Bv, Lv = seg.shape
    wgt = np.zeros((Bv, Lv), dtype=np.float32)
    for b in range(Bv):
        counts = np.bincount(seg[b][valid[b]], minlength=Lv + 1).astype(np.float32)
        inv = 1.0 / np.maximum(counts, 1.0)
        wgt[b] = np.where(valid[b] & (seg[b] < n_words), inv[np.minimum(seg[b], Lv)], 0.0)
    return wgt


# ---------------------------------------------------------------------------
# Device kernel (raw bass)
# ---------------------------------------------------------------------------

def _chunk_layout(S, T, block_ok):
    """Per-chunk packed column layout: tok_h (H cols) then atw columns.

    block_ok: src chunks carry S membership cols, tgt chunks T cols.
    general:  every chunk carries S+T cols (src block then tgt block).
    Returns (offsets, widths, atw_meta, total) with atw_meta[c] =
    (src_off, tgt_off), None when that block is absent.
    """
    offs, widths, meta = [], [], []
    pos = 0
    for c in range(NCHUNK):
        is_src = c < N_SRC_CHUNKS
        if block_ok:
            wa = S if is_src else T
            meta.append((pos + H, None) if is_src else (None, pos + H))
        else:
            wa = S + T
            meta.append((pos + H, pos + H + S))
        offs.append(pos)
        widths.append(H + wa)
        pos += H + wa
    return offs, widths, meta, pos


@functools.lru_cache(maxsize=4)
def _build(S, T, block_ok):
    nc = bacc.Bacc("TRN2", debug=False, num_devices=N_CORES)
    offs, widths, meta, totw = _chunk_layout(S, T, block_ok)

    tok = nc.declare_dram_parameter("tok", [P, totw], BF16, isOutput=False)
    # wcat = [w_src (H) | w_tgt (H) | bias row (T copies of b)]
    wcat = nc.declare_dram_parameter("wcat", [1, 2 * H + T], BF16, isOutput=False)
    out = nc.declare_dram_parameter("out", [S, T], F32, isOutput=True)

    with ExitStack() as ctx:
        tok_sb = ctx.enter_context(nc.sbuf_tensor([P, totw], BF16))
        wcat_sb = ctx.enter_context(nc.sbuf_tensor([1, 2 * H + T], BF16))
        ones = ctx.enter_context(nc.sbuf_tensor([1, P], BF16))
        prod = ctx.enter_context(nc.sbuf_tensor([P, H], BF16))
        n_mm = NCHUNK if block_ok else 2 * NCHUNK
        u = ctx.enter_context(nc.sbuf_tensor([P, n_mm], F32))
        u_bf = ctx.enter_context(nc.sbuf_tensor([P, n_mm], BF16))
        out_sb = ctx.enter_context(nc.sbuf_tensor([S, T], F32))
        # PSUM: broadcast weights (2 banks each) + the output accumulator
        wsrc_ps = ctx.enter_context(nc.psum_tensor([P, H], F32))
        wtgt_ps = ctx.enter_context(nc.psum_tensor([P, H], F32))
        psum = ctx.enter_context(nc.psum_tensor([S, T], F32))

        dma_groups = [[c] for c in range(NCHUNK)]
        grp_of = {c: g for g, cs in enumerate(dma_groups) for c in cs}
        ck = [ctx.enter_context(nc.semaphore(name=f"ck{g}"))
              for g in range(len(dma_groups))]
        wcat_sem = ctx.enter_context(nc.semaphore())
        outd_sem = ctx.enter_context(nc.semaphore())
        ones_sem = ctx.enter_context(nc.semaphore())
        wb_sem = ctx.enter_context(nc.semaphore())
        u_sem = ctx.enter_context(nc.semaphore())
        ub_sem = ctx.enter_context(nc.semaphore())
        mm_sem = ctx.enter_context(nc.semaphore())
        cp_sem = ctx.enter_context(nc.semaphore())
        sems = ck + [wcat_sem, outd_sem, ones_sem, wb_sem,
                     u_sem, ub_sem, mm_sem, cp_sem]
        sem_nums = sorted(s.num for s in sems)
        assert sem_nums[-1] - sem_nums[0] == len(sems) - 1, sem_nums

        # per-chunk matmul plan: (c, kind, atw col offset) in chain order
        mm_plan = []
        for c in range(NCHUNK):
            src_off, tgt_off = meta[c]
            if src_off is not None:
                mm_plan.append((c, "src", src_off))
            if tgt_off is not None:
                mm_plan.append((c, "tgt", tgt_off))

        with nc.Block(no_gpsimd_drain=True) as block:

            @block.sync
            def _(sync):
                # weights row first (tiny; everything upstream needs it)
                sync.dma_start(out=wcat_sb[:, :], in_=wcat[:, :]).then_inc(
                    wcat_sem, 16)
                for g, cs in enumerate(dma_groups):
                    sl = slice(offs[cs[0]], offs[cs[-1]] + widths[cs[-1]])
                    sync.dma_start(out=tok_sb[:, sl], in_=tok[:, sl]).then_inc(
                        ck[g], 16)
                # out store (issued here once the DVE's psum->sbuf copy lands;
                # ACT stays entirely unused, which drops its table load)
                sync.wait_ge(cp_sem, 1)
                sync.dma_start(out=out[:, :], in_=out_sb[:, :]).then_inc(
                    outd_sem, 16)
                sync.wait_ge(outd_sem, 16)

            @block.vector
            def _(vector):
                nc.vector.memset(ones[:, :], 1.0).then_inc(ones_sem, 1)
                seen_g = -1
                for i, (c, kind, _aoff) in enumerate(mm_plan):
                    if grp_of[c] != seen_g:
                        vector.wait_ge(ck[grp_of[c]], 16)
                        seen_g = grp_of[c]
                    vector.wait_ge(wb_sem, 1 if kind == "src" else 2)
                    wps = wsrc_ps if kind == "src" else wtgt_ps
                    ucol = u[:, i:i + 1]
                    nc.vector.affine_mul_reduce(
                        out=prod[:, :],
                        accum_out=ucol,
                        in0=tok_sb[:, offs[c]:offs[c] + H],
                        in1=wps[:, :],
                        scale=1.0, bias=0.0).then_inc(u_sem, 1)
                    # custom-op accum flush: the race detector wants explicit
                    # sync even same-engine (HW drains anyway)
                    vector.wait_ge(u_sem, i + 1)
                    # cast u to bf16; the matmul reads it through a stride-0
                    # broadcast AP, so no [P,T] tile is materialized
                    nc.vector.tensor_copy(
                        u_bf[:, i:i + 1], ucol).then_inc(ub_sem, 1)
                # epilogue: psum -> sbuf (DMA cannot read PSUM)
                vector.wait_ge(mm_sem, 1)
                nc.vector.tensor_copy(out_sb[:, :], psum[:, :]).then_inc(
                    cp_sem, 1)

            @block.tensor
            def _(tensor):
                tensor.wait_ge(ones_sem, 1)
                tensor.wait_ge(wcat_sem, 16)
                # broadcast w rows down the partitions: [P,H] = ones^T @ w_row
                for wi, wps in ((0, wsrc_ps), (1, wtgt_ps)):
                    for j0, j1 in ((0, 512), (512, H)):
                        mm = nc.tensor.matmul(
                            wps[:, j0:j1], ones[:, :P],
                            wcat_sb[:, wi * H + j0:wi * H + j1],
                            start=True, stop=True)
                        if j1 == H:
                            mm.then_inc(wb_sem, 1)
                # bias enters the output accumulation group first
                nc.tensor.matmul(
                    psum[:, :], ones[:, :S], wcat_sb[:, 2 * H:2 * H + T],
                    start=True, stop=False)
                for i, (c, kind, aoff) in enumerate(mm_plan):
                    tensor.wait_ge(ub_sem, i + 1)
                    last = i == len(mm_plan) - 1
                    if kind == "src":
                        mm = nc.tensor.matmul(
                            psum[:, :],
                            tok_sb[:, aoff:aoff + S],
                            u_bf[:, i:i + 1].broadcast_to([P, T]),
                            start=False, stop=last)
                    else:
                        mm = nc.tensor.matmul(
                            psum[:, :],
                            u_bf[:, i:i + 1].broadcast_to([P, S]),
                            tok_sb[:, aoff:aoff + T],
                            start=False, stop=last)
                    if last:
                        mm.then_inc(mm_sem, 1)

        # after the Block's exit barrier every engine has synced on all sem
        # updates, so a single range-clear resets them for re-execution
        nc.sync.sem_clear(range(sem_nums[0], sem_nums[-1] + 1))

        nc.compile()
    return nc


# ---------------------------------------------------------------------------
# Host wrapper
# ---------------------------------------------------------------------------

def _prep(inputs):
    tok_h = np.ascontiguousarray(np.asarray(inputs["tok_h"], dtype=np.float32))
    mask = np.asarray(inputs["attention_mask"])
    swid = np.asarray(inputs["source_word_ids"])
    twid = np.asarray(inputs["target_word_ids"])
    W = np.asarray(inputs["W"], dtype=np.float32)
    b = np.asarray(inputs["b"], dtype=np.float32)
    S = int(np.asarray(inputs["S"]))
    T = int(np.asarray(inputs["T"]))

    Bv, Lv, Hv = tok_h.shape
    assert (Bv, Lv, Hv) == (B, L, H), f"unexpected tok_h shape {tok_h.shape}"
    assert swid.shape == (B, L_SRC) and twid.shape == (B, L_TGT)
    assert S <= P and T <= P

    NW = S + T
    combined = np.concatenate([swid, twid], axis=1).astype(np.int64)
    seg, valid = _segments(combined, mask, NW)
    wgt = _seg_weights(seg, valid, NW)

    src_tok_seg = seg[:, :L_SRC][valid[:, :L_SRC]]
    tgt_tok_seg = seg[:, L_SRC:][valid[:, L_SRC:]]
    block_ok = bool(
        (src_tok_seg < S).all()
        and (tgt_tok_seg >= S).all() and (tgt_tok_seg < NW).all()
    )

    offs, widths, meta, totw = _chunk_layout(S, T, block_ok)

    # wcat row: [w_src | w_tgt | bias row]
    wcat = np.empty((1, 2 * H + T), dtype=NPBF16)
    wcat[0, 0:H] = W[:H, 0].astype(NPBF16)
    wcat[0, H:2 * H] = W[H:2 * H, 0].astype(NPBF16)
    wcat[0, 2 * H:] = NPBF16(b.reshape(-1)[0])

    tok_bf = tok_h.astype(NPBF16)
    in_maps = []
    for i in range(N_CORES):
        bi = i % B
        tokpack = np.zeros((P, totw), dtype=NPBF16)
        for c in range(NCHUNK):
            tsl = slice(c * P, (c + 1) * P)
            tokpack[:, offs[c]:offs[c] + H] = tok_bf[bi, tsl, :]
            segc = seg[bi, tsl]
            wgtc = wgt[bi, tsl]
            src_off, tgt_off = meta[c]
            # atw[tok, word] = wgt * (seg == word), split by block
            if src_off is not None:
                atw = np.zeros((P, S), dtype=np.float32)
                ok = segc < S
                atw[np.arange(P)[ok], segc[ok]] = wgtc[ok]
                tokpack[:, src_off:src_off + S] = atw.astype(NPBF16)
            if tgt_off is not None:
                atw = np.zeros((P, T), dtype=np.float32)
                ok = (segc >= S) & (segc < NW)
                atw[np.arange(P)[ok], segc[ok] - S] = wgtc[ok]
                tokpack[:, tgt_off:tgt_off + T] = atw.astype(NPBF16)
        in_maps.append({"tok": tokpack, "wcat": wcat})
    return S, T, block_ok, in_maps


def kernel(**inputs):
    S, T, block_ok, in_maps = _prep(inputs)
    nc = _build(S, T, block_ok)
    res = run_bass_kernel_spmd(nc, in_maps, core_ids=list(range(N_CORES)))
    return np.stack([res.results[i]["out"] for i in range(B)], axis=0)
